# revision 1
# baseline (speedup 1.0000x reference)
"""Additive (Bahdanau) attention kernel for Trainium2, 8 NeuronCores.

Problem shapes (hardcoded): B=8, TQ=128, TV=256, D=512, U=256.
Sharding: data-parallel over batch B -> one batch element per core.

Per-core algorithm (all on-chip after the initial DMAs):
  w1vT[u,v]  = (values @ W1)^T           via PE (K=d chunks), bias b1 folded later
  w2qT[u,q]  = (query  @ W2)^T + (b1+b2) via PE + ACT bias
  for each q (blocks of 16):
      pre[u,(q,c,v)] = w1vT[u,(c,v)] + w2qT[u,(c,q)]   (DVE/GPSIMD broadcast add)
      feat = tanh(pre)                                  (ACT, one big-FD instr)
      score[q,:] += Vwin_c(q)^T @ feat_c                (PE one-hot sliding window)
  attn = exp(score) (no max-sub needed; |score|<=13), rowsum via accum_out
  context = attn^T-transpose -> (attnT^T @ values) scaled by 1/rowsum
bv is dropped: softmax is invariant to a constant shift.
"""
import sys
import numpy as np

if '/opt/trn_rl_repo' not in sys.path:
    sys.path.insert(0, '/opt/trn_rl_repo')

B, TQ, TV, D, U = 8, 128, 256, 512, 256
P = 128          # partitions
KD = D // P      # 4 k-chunks over d
CU = U // P      # 2 chunks over u
CV = TV // P     # 2 chunks over v
BQ = 16          # q-block size
NBLK = TQ // BQ  # 8 blocks
N_GP = 3         # q's per block computed on GPSIMD (rest on DVE)

_compiled = None


def _build():
    import concourse.bass as bass
    import concourse.tile as tile
    from concourse import bacc, mybir

    f32 = mybir.dt.float32
    AF = mybir.ActivationFunctionType

    nc = bacc.Bacc("TRN2", target_bir_lowering=False, debug=False,
                   enable_asserts=True, num_devices=B)

    W1_d = nc.dram_tensor("W1", [D, U], f32, kind="ExternalInput").ap()
    W2_d = nc.dram_tensor("W2", [D, U], f32, kind="ExternalInput").ap()
    QT_d = nc.dram_tensor("QT", [D, TQ], f32, kind="ExternalInput").ap()
    VT_d = nc.dram_tensor("VT", [D, TV], f32, kind="ExternalInput").ap()
    VAL_d = nc.dram_tensor("VAL", [TV, D], f32, kind="ExternalInput").ap()
    VW_d = nc.dram_tensor("VW", [P, CU, 255], f32, kind="ExternalInput").ap()
    B12_d = nc.dram_tensor("B12", [P, CU], f32, kind="ExternalInput").ap()
    ID_d = nc.dram_tensor("ID", [P, P], f32, kind="ExternalInput").ap()
    OUT_d = nc.dram_tensor("OUT", [TQ, D], f32, kind="ExternalOutput").ap()

    with tile.TileContext(nc) as tc:
        with (
            tc.tile_pool(name="cst", bufs=1) as cst,
            tc.tile_pool(name="pre_p", bufs=3) as pre_p,
            tc.tile_pool(name="sm", bufs=1) as sm,
            tc.tile_pool(name="ps", bufs=1, space=bass.MemorySpace.PSUM) as ps,
        ):
            # ---- constant / input tiles ----
            w1 = cst.tile([P, KD, U], f32, tag="w1")
            nc.sync.dma_start(w1[:], W1_d.rearrange("(k p) u -> p k u", p=P))
            w2 = cst.tile([P, KD, U], f32, tag="w2")
            nc.sync.dma_start(w2[:], W2_d.rearrange("(k p) u -> p k u", p=P))
            qt = cst.tile([P, KD, TQ], f32, tag="qt")
            nc.sync.dma_start(qt[:], QT_d.rearrange("(k p) q -> p k q", p=P))
            vt = cst.tile([P, KD, TV], f32, tag="vt")
            nc.sync.dma_start(vt[:], VT_d.rearrange("(k p) v -> p k v", p=P))
            val = cst.tile([P, CV, D], f32, tag="val")
            nc.sync.dma_start(val[:], VAL_d.rearrange("(c p) d -> p c d", p=P))
            vw = cst.tile([P, CU, 255], f32, tag="vw")
            nc.sync.dma_start(vw[:], VW_d)
            b12 = cst.tile([P, CU], f32, tag="b12")
            nc.sync.dma_start(b12[:], B12_d)
            idt = cst.tile([P, P], f32, tag="idt")
            nc.sync.dma_start(idt[:], ID_d)

            # ---- projections ----
            psW1 = ps.tile([P, CU, TV], f32, tag="psW1")   # one bank
            for c in range(CU):
                for k in range(KD):
                    nc.tensor.matmul(psW1[:, c, :],
                                     w1[:, k, c * P:(c + 1) * P],
                                     vt[:, k, :],
                                     start=(k == 0), stop=(k == KD - 1))
            w1vT = cst.tile([P, CU, TV], f32, tag="w1vT")
            nc.scalar.copy(w1vT[:], psW1[:])

            psW2 = ps.tile([P, CU, TQ], f32, tag="psW2")   # half bank
            for c in range(CU):
                for k in range(KD):
                    nc.tensor.matmul(psW2[:, c, :],
                                     w2[:, k, c * P:(c + 1) * P],
                                     qt[:, k, :],
                                     start=(k == 0), stop=(k == KD - 1))
            w2qT = cst.tile([P, CU, TQ], f32, tag="w2qT")
            for c in range(CU):
                nc.scalar.activation(w2qT[:, c, :], psW2[:, c, :],
                                     AF.Identity, bias=b12[:, c:c + 1])

            # ---- score phase ----
            score_ps = ps.tile([P, TV], f32, tag="score")  # half bank
            n_mm = TQ * CU
            mm = 0
            for blk in range(NBLK):
                q0 = blk * BQ
                pre = pre_p.tile([P, BQ, CU, TV], f32, tag="pre")
                n_dve = BQ - N_GP
                in0 = w1vT[:].unsqueeze(1).broadcast_to([P, n_dve, CU, TV])
                in1 = (w2qT[:, :, q0:q0 + n_dve]
                       .rearrange("p c q -> p q c")
                       .unsqueeze(3).broadcast_to([P, n_dve, CU, TV]))
                nc.vector.tensor_add(pre[:, 0:n_dve, :, :], in0, in1)
                if N_GP:
                    g0 = q0 + n_dve
                    in0g = w1vT[:].unsqueeze(1).broadcast_to([P, N_GP, CU, TV])
                    in1g = (w2qT[:, :, g0:g0 + N_GP]
                            .rearrange("p c q -> p q c")
                            .unsqueeze(3).broadcast_to([P, N_GP, CU, TV]))
                    nc.gpsimd.tensor_add(pre[:, n_dve:BQ, :, :], in0g, in1g)
                nc.scalar.activation(pre[:], pre[:], AF.Tanh)
                for ql in range(BQ):
                    q = q0 + ql
                    for c in range(CU):
                        nc.tensor.matmul(score_ps[:],
                                         vw[:, c, 127 - q:255 - q],
                                         pre[:, ql, c, :],
                                         start=(mm == 0), stop=(mm == n_mm - 1))
                        mm += 1

            # ---- softmax (no max subtraction; scores are bounded) ----
            att = sm.tile([P, TV], f32, tag="att")
            sums = sm.tile([P, 2], f32, tag="sums")
            nc.scalar.activation(att[:], score_ps[:], AF.Exp,
                                 accum_out=sums[:, 0:1])
            nc.vector.reciprocal(sums[:, 1:2], sums[:, 0:1])

            # ---- context = softmax(score) @ values ----
            psT = ps.tile([P, CV, P], f32, tag="psT")      # half bank
            for c in range(CV):
                nc.tensor.transpose(psT[:, c, :], att[:, c * P:(c + 1) * P],
                                    idt[:])
            attnT = sm.tile([P, CV, P], f32, tag="attnT")
            nc.vector.tensor_copy(attnT[:], psT[:])

            ctx_ps = ps.tile([P, D], f32, tag="ctx")       # one bank
            for c in range(CV):
                nc.tensor.matmul(ctx_ps[:], attnT[:, c, :], val[:, c, :],
                                 start=(c == 0), stop=(c == CV - 1))
            ctx = sm.tile([P, D], f32, tag="ctxsb")
            nc.vector.tensor_scalar_mul(ctx[:], ctx_ps[:], sums[:, 1:2])
            nc.sync.dma_start(OUT_d, ctx[:])

    nc.compile()
    return nc


def _prep_shared(W1, b1, W2, b2, V, bv):
    Vw = np.zeros((P, CU, 255), np.float32)
    for c in range(CU):
        Vw[:, c, 127] = V[c * P:(c + 1) * P, 0]
    b12 = (b1 + b2).astype(np.float32).reshape(CU, P).T.copy()
    ident = np.eye(P, dtype=np.float32)
    return {
        "W1": np.ascontiguousarray(W1, np.float32),
        "W2": np.ascontiguousarray(W2, np.float32),
        "VW": Vw,
        "B12": np.ascontiguousarray(b12),
        "ID": ident,
    }


def kernel(query, values, W1, b1, W2, b2, V, bv, _trace=False, _tmpdir=None):
    global _compiled
    from concourse.bass_utils import run_bass_kernel_spmd

    query = np.asarray(query, np.float32)
    values = np.asarray(values, np.float32)
    shared = _prep_shared(np.asarray(W1), np.asarray(b1), np.asarray(W2),
                          np.asarray(b2), np.asarray(V), np.asarray(bv))

    if _compiled is None:
        _compiled = _build()
    nc = _compiled

    in_maps = []
    for i in range(B):
        m = dict(shared)
        m["QT"] = np.ascontiguousarray(query[i].T)
        m["VT"] = np.ascontiguousarray(values[i].T)
        m["VAL"] = np.ascontiguousarray(values[i])
        in_maps.append(m)

    kw = {}
    if _trace:
        kw.update(trace=True, tmpdir=_tmpdir)
    res = run_bass_kernel_spmd(nc, in_maps, core_ids=list(range(B)), **kw)
    out = np.stack([res.results[i]["OUT"] for i in range(B)], axis=0)
    if _trace:
        kernel._last_trace = res
    return out
